# revision 1
# baseline (speedup 1.0000x reference)
"""GAT message-passing kernel for Trainium2 (8 NeuronCores, SPMD).

Strategy (dst-partitioned, no collectives):
  - Sort edges by dst on host; core c owns dst nodes [c*6250, (c+1)*6250).
  - Phase A (replicated): feat_aug = h @ [fc_w | A_l | A_r] written to a DRAM
    table G (bf16 rows, feat stored (f,h)-major + el); per-core er table.
  - Phase B: per 128-node window (processed in pairs), batch-gather edge rows
    via InstDMAGatherAnt (int16 idx; lo/hi table split for the 32768 index
    limit), compute w = max(exp(e), exp(0.2 e)) (== exp(leaky_relu(e))),
    build a one-hot selection matrix per chunk and accumulate messages +
    softmax denominators into PSUM with TensorE matmuls (single-pass softmax).
  - Phase C: rst = msg/z; out linear (PE transpose + matmul) + folded bias
    (gat_bias @ out_w + out_b) + layernorm; DMA out.
"""
import os
import numpy as np

import concourse.bass as bass
import concourse.bacc as bacc
import concourse.mybir as mybir
import concourse.tile as tile
import concourse.bass_utils as bu
from concourse.bass_utils import run_bass_kernel_spmd
from concourse.masks import make_identity
from concourse.tile_rust import add_dep_helper
from concourse import ap_utils
from concourse._compat import exact_div

# ---------------- constants ----------------
N, E, F, H = 50000, 800000, 64, 4
HF = H * F
NCORES = 8
NPC = N // NCORES            # 6250
P = 128
NWIN = (NPC + P - 1) // P    # 49
LO = 32768                   # int16 index split point
GROWS = 50176                # 49*1024, padded node count for phase A supertiles
GSTRIDE = 384                # bf16 elems per G row (768B, mult of 256B)
GROW_USED = 260              # feat 256 (f,h)-major | el 4
ERROWS = 6272                # 49*128
ERSTRIDE = 128               # bf16 elems per er row (256B)
LN_EPS = 1e-5
NEG = 0.2
F32 = mybir.dt.float32
BF16 = mybir.dt.bfloat16
I16 = mybir.dt.int16

# ---------------- walrus DGE patch (vector-indirect DMA support) ------------
_DGE_FLAG = "--dge-levels=vector_dynamic_offsets,dst_reduce"
_orig_bvo = bu.bir_verify_and_optimise

def _patched_bvo(tmpdir, inp="bir.json", outp="file.neff", arch=None, *, dve_root=None):
    orig_run = bu.run_command
    def run2(cmd, **kw):
        cmd = list(cmd)
        cmd.insert(1, _DGE_FLAG)
        return orig_run(cmd, **kw)
    bu.run_command = run2
    try:
        return _orig_bvo(tmpdir, inp, outp, arch, dve_root=dve_root)
    finally:
        bu.run_command = orig_run

bu.bir_verify_and_optimise = _patched_bvo


def dma_gather_relaxed(eng, out_ap, in_ap, idxs_ap, num_idxs_reg, num_idxs, elem_size,
                       elem_step, queue_num=0, single_packet=False):
    """nc.gpsimd.dma_gather minus the elem_size%256 assert (stride must still
    be a multiple of 256B; read length per row may be arbitrary)."""
    assert idxs_ap.dtype == I16
    assert in_ap.space == bass.MemorySpace.DRAM
    assert idxs_ap.space == bass.MemorySpace.SBUF
    assert out_ap.space == bass.MemorySpace.SBUF
    assert ap_utils.ap_is_contiguous(in_ap.ap[1:])
    assert ap_utils.ap_is_contiguous(out_ap.ap[1:])
    assert ap_utils.ap_is_contiguous(idxs_ap.ap[1:])
    assert in_ap.ap[-1][1] == out_ap.ap[-1][1] == elem_size
    assert out_ap.ap[0][1] * out_ap.ap[1][1] == num_idxs, (out_ap.ap, num_idxs)
    assert in_ap.ap[0][0] == elem_step
    stride_bytes = elem_step * mybir.dt.size(in_ap.dtype)
    stride_bytes_256 = exact_div(stride_bytes, 256)
    assert stride_bytes_256 < 256
    _in_ap = eng.lower_ap_dma(in_ap, for_custom_bir_dma=True)
    _idxs_ap = eng.lower_ap(idxs_ap)
    _out_ap = eng.lower_ap(out_ap)
    return eng.add_instruction(
        mybir.InstDMAGatherAnt(
            name=eng.bass.get_next_instruction_name(),
            ins=[*_in_ap, _idxs_ap, eng.lower_val_access(num_idxs_reg)],
            outs=[_out_ap],
            transpose=False,
            num_idxs=num_idxs,
            elem_size=elem_size,
            stride_bytes_256=stride_bytes_256,
            gen_mode=0,
            single_packet=single_packet,
            queue_num=queue_num,
            sbuf_tokens_per_rank=0,
            sbuf_free_dim_per_rank=0,
            sbuf_free_dim_pad_per_rank=0,
            sbuf_byte_offset=0,
        )
    )


def _pieces(j0, j1, maxc=8):
    out = []
    while j0 < j1:
        out.append((j0, min(j0 + maxc, j1)))
        j0 = min(j0 + maxc, j1)
    return out


def _groups():
    gs = [(w, w + 1) for w in range(0, NWIN - 1, 2)]
    if NWIN % 2:
        gs.append((NWIN - 1,))
    return gs


def build_program(CL, CH):
    CPW = CL + CH
    nc = bacc.Bacc("TRN2", target_bir_lowering=False, debug=False, num_devices=NCORES)

    hT_p = nc.declare_dram_parameter("hT", [F, GROWS], BF16, isOutput=False)
    hTo_p = nc.declare_dram_parameter("hTo", [F, ERROWS], BF16, isOutput=False)
    Waug_p = nc.declare_dram_parameter("Waug", [F, 264], BF16, isOutput=False)
    outw_p = nc.declare_dram_parameter("outw", [HF, F], F32, isOutput=False)  # (f,h)-permuted
    vecs_p = nc.declare_dram_parameter("vecs", [P, 3, F], F32, isOutput=False)
    srcq_p = nc.declare_dram_parameter("srcq", [P, NWIN * CPW * 8], I16, isOutput=False)
    dstl_p = nc.declare_dram_parameter("dstl", [P, NWIN * CPW * 8], I16, isOutput=False)
    dstf_p = nc.declare_dram_parameter("dstf", [P, NWIN * CPW], F32, isOutput=False)
    out_p = nc.declare_dram_parameter("out", [NWIN * P, F], F32, isOutput=True)

    G = nc.dram_tensor("G", [GROWS, GSTRIDE], BF16)
    ERL = nc.dram_tensor("ERL", [ERROWS, ERSTRIDE], BF16)

    with tile.TileContext(nc) as tc:
        with tc.tile_pool(name="const", bufs=1) as cp:
            iota_f = cp.tile([P, 2 * CPW, P], BF16)
            nc.gpsimd.iota(iota_f[:], pattern=[[0, 2 * CPW], [1, P]], base=0,
                           channel_multiplier=0, allow_small_or_imprecise_dtypes=True)
            ident = cp.tile([P, P], F32)
            make_identity(nc, ident[:])
            Waug_t = cp.tile([F, 264], BF16)
            nc.sync.dma_start(out=Waug_t[:], in_=Waug_p[:])
            outw_t = cp.tile([P, 2, F], F32)
            nc.sync.dma_start(out=outw_t[:], in_=outw_p[:].rearrange("(k p) f -> p k f", p=P))
            vecs_t = cp.tile([P, 3, F], F32)
            nc.sync.dma_start(out=vecs_t[:], in_=vecs_p[:])
            srcq_t = cp.tile([P, NWIN * CPW * 8], I16)
            nc.sync.dma_start(out=srcq_t[:], in_=srcq_p[:])
            dstl_t = cp.tile([P, NWIN * CPW * 8], I16)
            nc.sync.dma_start(out=dstl_t[:], in_=dstl_p[:])
            dstf_t = cp.tile([P, NWIN * CPW], F32)
            nc.sync.dma_start(out=dstf_t[:], in_=dstf_p[:])

            # cached num_idxs registers
            _regs = {}
            def nreg(v):
                if v not in _regs:
                    _regs[v] = nc.gpsimd.to_reg(v)
                return _regs[v]

            lo_writes, er_writes, hi_writes = [], [], []
            # B-phase SBUF pools opened FIRST so their addresses don't overlap
            # phase-A staging (avoids false WAR deps stalling the first gathers)
            pools_b = [tc.tile_pool(name="phbl", bufs=5), tc.tile_pool(name="phb", bufs=3),
                       tc.tile_pool(name="phsel", bufs=2), tc.tile_pool(name="phc", bufs=4)]
            pbl, pb, psel, pc = [p.__enter__() for p in pools_b]
            # ---------------- phase A: G table (lo rows first, then er, then hi) ----------------
            with tc.tile_pool(name="pha", bufs=3) as pa, \
                 tc.tile_pool(name="phaps", bufs=4, space="PSUM") as pap:
                def do_supertile(st, sink):
                    ht = pa.tile([F, 1024], BF16, tag="ht")
                    nc.sync.dma_start(out=ht[:], in_=hT_p[:, st * 1024:(st + 1) * 1024])
                    stg = pa.tile([P, 8, GROW_USED], BF16, tag="stg")
                    for t in range(8):
                        ps = pap.tile([P, 264], F32, tag="psA")
                        nc.tensor.matmul(out=ps[:], lhsT=ht[:, t * P:(t + 1) * P],
                                         rhs=Waug_t[:], start=True, stop=True)
                        if t % 2 == 0:
                            nc.vector.tensor_copy(
                                out=stg[:, t, 0:256].rearrange("p (f h) -> p f h", f=F),
                                in_=ps[:, 0:256].rearrange("p (h f) -> p f h", h=H))
                            nc.scalar.copy(out=stg[:, t, 256:260], in_=ps[:, 256:260])
                        else:
                            nc.scalar.copy(
                                out=stg[:, t, 0:256].rearrange("p (f h) -> p f h", f=F),
                                in_=ps[:, 0:256].rearrange("p (h f) -> p f h", h=H))
                            nc.vector.tensor_copy(out=stg[:, t, 256:260], in_=ps[:, 256:260])
                    wi = nc.sync.dma_start(
                        out=G[st * 1024:(st + 1) * 1024, 0:GROW_USED].rearrange(
                            "(s p) c -> p s c", p=P),
                        in_=stg[:])
                    sink.append(wi)
                # ---------------- phase A2 first: er table (own nodes) ----------------
                for sg in range(7):
                    hb = pa.tile([F, 896], BF16, tag="hb")
                    nc.sync.dma_start(out=hb[:], in_=hTo_p[:, sg * 896:(sg + 1) * 896])
                    stg2 = pa.tile([P, 7, 4], BF16, tag="stg2")
                    for t in range(7):
                        ps2 = pap.tile([P, 4], F32, tag="psA2")
                        nc.tensor.matmul(out=ps2[:], lhsT=hb[:, t * P:(t + 1) * P],
                                         rhs=Waug_t[:, 260:264], start=True, stop=True)
                        nc.scalar.copy(out=stg2[:, t, :], in_=ps2[:])
                    wi = nc.sync.dma_start(
                        out=ERL[sg * 896:(sg + 1) * 896, 0:4].rearrange(
                            "(s p) c -> p s c", p=P),
                        in_=stg2[:])
                    er_writes.append(wi)
                join_er = nc.gpsimd.nop(nofuse=True)
                for wi in er_writes:
                    add_dep_helper(join_er.ins, wi.ins, reason="join_er waits on er writes")
                for st in range(LO // 1024):
                    do_supertile(st, lo_writes)
                join_lo = nc.gpsimd.nop(nofuse=True)
                for wi in lo_writes:
                    add_dep_helper(join_lo.ins, wi.ins, reason="join_lo waits on lo writes")
                for st in range(LO // 1024, GROWS // 1024):
                    do_supertile(st, hi_writes)

            join_hi_box = []

            def get_join_hi():
                if not join_hi_box:
                    jh = nc.gpsimd.nop(nofuse=True)
                    for wi in hi_writes:
                        add_dep_helper(jh.ins, wi.ins, reason="join_hi waits on hi writes")
                    join_hi_box.append(jh)
                return join_hi_box[0]

            # ---------------- phase B + C ----------------
            with tc.tile_pool(name="phbps", bufs=4, space="PSUM") as pwp, \
                 tc.tile_pool(name="phcps", bufs=2, space="PSUM") as pcp:
                groups = _groups()
                gb = []
                _acc = 0
                for grp in groups:
                    gb.append(_acc)
                    _acc += len(grp) * CPW
                PRE = 3  # groups whose lo/er gathers are prefetched ahead
                Xts, ERts = {}, {}

                def emit_lo_er(gi):
                    grp = groups[gi]
                    W = len(grp)
                    GC = W * CPW
                    b8 = gb[gi] * 8
                    lo_ch = W * CL
                    Xlo = pbl.tile([P, 2 * CL, GROW_USED], BF16, tag="Xlo")
                    ER = pbl.tile([P, 2 * CPW, 4], BF16, tag="ER")
                    Xts[gi], ERts[gi] = Xlo, ER
                    for (j0, j1) in _pieces(0, lo_ch):
                        g = dma_gather_relaxed(
                            nc.gpsimd, out_ap=Xlo[:, j0:j1, :], in_ap=G[0:LO, 0:GROW_USED],
                            idxs_ap=srcq_t[:, b8 + j0 * 8: b8 + j1 * 8],
                            num_idxs_reg=nreg((j1 - j0) * P),
                            num_idxs=(j1 - j0) * P, elem_size=GROW_USED,
                            elem_step=GSTRIDE)
                        add_dep_helper(g.ins, join_lo.ins, reason="gather after tables")
                    for (j0, j1) in _pieces(0, GC):
                        g = dma_gather_relaxed(
                            nc.gpsimd, out_ap=ER[:, j0:j1, :], in_ap=ERL[:, 0:4],
                            idxs_ap=dstl_t[:, b8 + j0 * 8: b8 + j1 * 8],
                            num_idxs_reg=nreg((j1 - j0) * P),
                            num_idxs=(j1 - j0) * P, elem_size=4, elem_step=ERSTRIDE)
                        add_dep_helper(g.ins, join_er.ins, reason="gather after tables")

                def complete(gi):
                    grp = groups[gi]
                    W = len(grp)
                    GC = W * CPW
                    gbase = gb[gi]
                    b8 = gbase * 8
                    lo_ch = W * CL
                    Xlo, ER = Xts.pop(gi), ERts.pop(gi)
                    Xhi = pb.tile([P, 2 * CH, GROW_USED], BF16, tag="Xhi")
                    for (j0, j1) in _pieces(0, GC - lo_ch):
                        g = dma_gather_relaxed(
                            nc.gpsimd, out_ap=Xhi[:, j0:j1, :], in_ap=G[LO:GROWS, 0:GROW_USED],
                            idxs_ap=srcq_t[:, b8 + (lo_ch + j0) * 8: b8 + (lo_ch + j1) * 8],
                            num_idxs_reg=nreg((j1 - j0) * P),
                            num_idxs=(j1 - j0) * P, elem_size=GROW_USED,
                            elem_step=GSTRIDE)
                        add_dep_helper(g.ins, get_join_hi().ins, reason="gather after tables")
                    # per-half w pipeline so lo-chunk matmuls start before hi gathers land
                    wb = pb.tile([P, GC, 4], BF16, tag="wb")
                    sel = psel.tile([P, GC, P], BF16, tag="sel")
                    ew = pb.tile([P, GC, 4], F32, tag="ew")
                    w1 = pb.tile([P, GC, 4], F32, tag="w1")
                    w2 = pb.tile([P, GC, 4], F32, tag="w2")
                    for (h0, h1, Xh) in ((0, lo_ch, Xlo), (lo_ch, GC, Xhi)):
                        hs = slice(h0, h1)
                        xs = slice(0, h1 - h0)
                        nh = h1 - h0
                        nc.vector.tensor_tensor(out=ew[:, hs, :], in0=Xh[:, xs, 256:260],
                                                in1=ER[:, hs, :], op=mybir.AluOpType.add)
                        nc.scalar.activation(out=w1[:, hs, :], in_=ew[:, hs, :],
                                             func=mybir.ActivationFunctionType.Exp)
                        nc.scalar.activation(out=w2[:, hs, :], in_=ew[:, hs, :],
                                             func=mybir.ActivationFunctionType.Exp, scale=NEG)
                        nc.vector.tensor_tensor(out=wb[:, hs, :], in0=w1[:, hs, :],
                                                in1=w2[:, hs, :], op=mybir.AluOpType.max)
                        nc.vector.tensor_tensor(
                            out=sel[:, hs, :], in0=iota_f[:, h0:h1, :],
                            in1=dstf_t[:, gbase + h0:gbase + h1, None].to_broadcast([P, nh, P]),
                            op=mybir.AluOpType.is_equal)
                        # fold w into features in place ((f,h)-packed broadcast)
                        nc.vector.tensor_tensor(
                            out=Xh[:, xs, 0:256].rearrange("p j (f h) -> p j f h", f=F),
                            in0=Xh[:, xs, 0:256].rearrange("p j (f h) -> p j f h", f=F),
                            in1=wb[:, hs, None, :].broadcast_to([P, nh, F, H]),
                            op=mybir.AluOpType.mult)
                        nc.vector.tensor_copy(out=Xh[:, xs, 256:260], in_=wb[:, hs, :])
                    # chunk -> window-slot ownership: [0]*CL+[1]*CL then [0]*CH+[1]*CH
                    own = []
                    for i in range(W):
                        own += [i] * CL
                    for i in range(W):
                        own += [i] * CH
                    first = {i: own.index(i) for i in range(W)}
                    last = {i: GC - 1 - own[::-1].index(i) for i in range(W)}
                    psws = []
                    for _pi in range(W):
                        psw_t = pwp.tile([P, GROW_USED], F32, tag="psw")
                        psws.append(psw_t)
                    for c in range(GC):
                        o = own[c]
                        rhs = Xlo[:, c, :] if c < lo_ch else Xhi[:, c - lo_ch, :]
                        nc.tensor.matmul(out=psws[o][:], lhsT=sel[:, c, :], rhs=rhs,
                                         start=(c == first[o]), stop=(c == last[o]),
                                         skip_group_check=True)
                    # ---- phase C (per window in group) ----
                    for wi_, wv in enumerate(grp):
                        psw = psws[wi_]
                        zs = pc.tile([P, 4], F32, tag="zs")
                        nc.vector.tensor_scalar(out=zs[:], in0=psw[:, 256:260],
                                                scalar1=1e-30, scalar2=None,
                                                op0=mybir.AluOpType.max)
                        zr = pc.tile([P, 4], F32, tag="zr")
                        nc.vector.reciprocal_approx_fast(out=zr[:], in_=zs[:])
                        rstn = pc.tile([P, HF], F32, tag="rstn")
                        nc.vector.tensor_tensor(out=rstn[:].rearrange("p (f h) -> p f h", f=F),
                                                in0=psw[:, 0:256].rearrange("p (f h) -> p f h", f=F),
                                                in1=zr[:, None, :].broadcast_to([P, F, H]),
                                                op=mybir.AluOpType.mult)
                        psx = pcp.tile([P, F], F32, tag="psx")
                        for half in range(2):
                            pst = pcp.tile([P, P], F32, tag="pst")
                            nc.tensor.transpose(out=pst[:], in_=rstn[:, half * P:(half + 1) * P],
                                                identity=ident[:])
                            rT = pc.tile([P, P], F32, tag="rT")
                            nc.vector.tensor_copy(out=rT[:], in_=pst[:])
                            nc.tensor.matmul(out=psx[:], lhsT=rT[:], rhs=outw_t[:, half, :],
                                             start=(half == 0), stop=(half == 1))
                        xt = pc.tile([P, F], F32, tag="xt")
                        s1 = pc.tile([P, 1], F32, tag="s1")
                        nc.vector.scalar_tensor_tensor(out=xt[:], in0=psx[:], scalar=1.0,
                                                       in1=vecs_t[:, 0, :],
                                                       op0=mybir.AluOpType.mult,
                                                       op1=mybir.AluOpType.add,
                                                       accum_out=s1[:])
                        negmu = pc.tile([P, 1], F32, tag="negmu")
                        nc.vector.tensor_scalar(out=negmu[:], in0=s1[:], scalar1=-1.0 / F,
                                                scalar2=None, op0=mybir.AluOpType.mult)
                        xc = pc.tile([P, F], F32, tag="xc")
                        nc.vector.tensor_scalar(out=xc[:], in0=xt[:], scalar1=negmu[:, 0:1],
                                                scalar2=None, op0=mybir.AluOpType.add)
                        scr = pc.tile([P, F], F32, tag="scr")
                        ss = pc.tile([P, 1], F32, tag="ss")
                        nc.vector.scalar_tensor_tensor(out=scr[:], in0=xc[:], scalar=1.0,
                                                       in1=xc[:], op0=mybir.AluOpType.mult,
                                                       op1=mybir.AluOpType.mult,
                                                       accum_out=ss[:])
                        v = pc.tile([P, 1], F32, tag="v")
                        nc.vector.tensor_scalar(out=v[:], in0=ss[:], scalar1=1.0 / F,
                                                scalar2=LN_EPS, op0=mybir.AluOpType.mult,
                                                op1=mybir.AluOpType.add)
                        sv = pc.tile([P, 1], F32, tag="sv")
                        nc.scalar.activation(out=sv[:], in_=v[:],
                                             func=mybir.ActivationFunctionType.Sqrt)
                        rstd = pc.tile([P, 1], F32, tag="rstd")
                        nc.vector.reciprocal_approx_fast(out=rstd[:], in_=sv[:])
                        t1 = pc.tile([P, F], F32, tag="t1")
                        nc.vector.scalar_tensor_tensor(out=t1[:], in0=xc[:], scalar=rstd[:, 0:1],
                                                       in1=vecs_t[:, 1, :],
                                                       op0=mybir.AluOpType.mult,
                                                       op1=mybir.AluOpType.mult)
                        y = pc.tile([P, F], F32, tag="y")
                        nc.vector.tensor_tensor(out=y[:], in0=t1[:], in1=vecs_t[:, 2, :],
                                                op=mybir.AluOpType.add)
                        nc.sync.dma_start(out=out_p[wv * P:(wv + 1) * P, :], in_=y[:])

                for gi in range(len(groups) + PRE):
                    if gi < len(groups):
                        emit_lo_er(gi)
                    if gi >= PRE:
                        complete(gi - PRE)
            for p in reversed(pools_b):
                p.__exit__(None, None, None)

    nc.compile()
    return nc


# ---------------- host side ----------------
def host_prep(h, src, dst, fc_w, attn_l, attn_r, gat_bias, out_w, out_b, ln_g, ln_b):
    h = np.ascontiguousarray(np.asarray(h, np.float32))
    src = np.asarray(src, np.int64)
    dst = np.asarray(dst, np.int64)
    fc_w = np.asarray(fc_w, np.float32)
    attn_l = np.asarray(attn_l, np.float32)
    attn_r = np.asarray(attn_r, np.float32)
    gat_bias = np.asarray(gat_bias, np.float32)
    out_w = np.asarray(out_w, np.float32)
    out_b = np.asarray(out_b, np.float32)
    ln_g = np.asarray(ln_g, np.float32)
    ln_b = np.asarray(ln_b, np.float32)

    A_l = np.einsum('khf,hf->kh', fc_w.reshape(F, H, F), attn_l).astype(np.float32)
    A_r = np.einsum('khf,hf->kh', fc_w.reshape(F, H, F), attn_r).astype(np.float32)
    Waug = np.ascontiguousarray(np.concatenate([fc_w, A_l, A_r], axis=1))  # [64, 264]
    bias2 = (gat_bias @ out_w + out_b).astype(np.float32)                  # [64]
    # out_w permuted to (f,h)-major rows to match the G feat layout
    outw_perm = np.ascontiguousarray(
        out_w.reshape(H, F, F).transpose(1, 0, 2).reshape(HF, F))

    import ml_dtypes
    hT = np.zeros((F, GROWS), ml_dtypes.bfloat16)
    hT[:, :N] = h.T.astype(ml_dtypes.bfloat16)
    hTo = np.zeros((NCORES, F, ERROWS), ml_dtypes.bfloat16)
    for c in range(NCORES):
        hTo[c, :, :NPC] = h[c * NPC:(c + 1) * NPC].T.astype(ml_dtypes.bfloat16)

    vecs = np.zeros((P, 3, F), np.float32)
    vecs[:, 0, :] = bias2
    vecs[:, 1, :] = ln_g
    vecs[:, 2, :] = ln_b

    # sort edges by dst
    order = np.argsort(dst, kind='stable')
    ssrc = src[order]
    sdst = dst[order]
    core_of = sdst // NPC
    loc = sdst - core_of * NPC
    win = loc // P
    dloc = (loc - win * P).astype(np.float32)
    gw = core_of * NWIN + win
    counts = np.bincount(gw, minlength=NCORES * NWIN)
    starts = np.zeros(NCORES * NWIN + 1, np.int64)
    np.cumsum(counts, out=starts[1:])

    lomask = ssrc < LO
    CL = CH = 1
    for g in range(NCORES * NWIN):
        sl = slice(starts[g], starts[g + 1])
        nlo = int(lomask[sl].sum())
        nhi = int(counts[g] - nlo)
        CL = max(CL, (nlo + P - 1) // P)
        CH = max(CH, (nhi + P - 1) // P)
    CPW = CL + CH

    groups = [(w, w + 1) for w in range(0, NWIN - 1, 2)]
    if NWIN % 2:
        groups.append((NWIN - 1,))

    srcq = np.zeros((NCORES, P, NWIN * CPW * 8), np.int16)
    dstl = np.zeros((NCORES, P, NWIN * CPW * 8), np.int16)
    dstf = np.full((NCORES, P, NWIN * CPW), 200.0, np.float32)

    for c in range(NCORES):
        gbase = 0
        for grp in groups:
            Wn = len(grp)
            GC = Wn * CPW
            sq = np.zeros(GC * P, np.int16)
            dl = np.zeros(GC * P, np.int16)
            df = np.full(GC * P, 200.0, np.float32)
            for i, wv in enumerate(grp):
                g = c * NWIN + wv
                sl = slice(starts[g], starts[g + 1])
                s_src = ssrc[sl]; s_dl = dloc[sl]
                m = lomask[sl]
                lo_src, lo_dl = s_src[m], s_dl[m]
                hi_src, hi_dl = s_src[~m], s_dl[~m]
                nlo, nhi = len(lo_src), len(hi_src)
                slo = i * CL * P           # lo section for window i
                shi = (Wn * CL + i * CH) * P
                sq[slo:slo + nlo] = lo_src
                sq[shi:shi + nhi] = hi_src - LO
                dl[slo:slo + nlo] = (wv * P + lo_dl).astype(np.int16)
                dl[shi:shi + nhi] = (wv * P + hi_dl).astype(np.int16)
                df[slo:slo + nlo] = lo_dl
                df[shi:shi + nhi] = hi_dl
            cols8 = slice(gbase * 8, gbase * 8 + GC * 8)
            srcq[c][:, cols8] = np.tile(sq.reshape(GC * 8, 16).T, (8, 1))
            dstl[c][:, cols8] = np.tile(dl.reshape(GC * 8, 16).T, (8, 1))
            dstf[c][:, gbase:gbase + GC] = df.reshape(GC, P).T
            gbase += GC

    small = dict(Waug=Waug.astype(ml_dtypes.bfloat16), outw=outw_perm, vecs=vecs)
    return hT, hTo, srcq, dstl, dstf, small, CL, CH


_prog_cache = {}

def kernel(**inputs):
    hT, hTo, srcq, dstl, dstf, small, CL, CH = host_prep(**inputs)
    key = (CL, CH)
    if key not in _prog_cache:
        _prog_cache[key] = build_program(CL, CH)
    nc = _prog_cache[key]
    in_maps = []
    for c in range(NCORES):
        in_maps.append({
            "hT": hT, "hTo": hTo[c], "Waug": small["Waug"], "outw": small["outw"],
            "vecs": small["vecs"], "srcq": srcq[c], "dstl": dstl[c], "dstf": dstf[c],
        })
    res = run_bass_kernel_spmd(nc, in_maps, list(range(NCORES)))
    out = np.concatenate([np.asarray(res.results[c]["out"])[:NPC] for c in range(NCORES)], axis=0)
    return out



# revision 12
# speedup vs baseline: 1.0624x; 1.0624x over previous
"""GAT message-passing kernel for Trainium2 (8 NeuronCores, SPMD).

Strategy (dst-partitioned, no collectives):
  - Sort edges by dst on host; core c owns dst nodes [c*6250, (c+1)*6250).
  - Phase A (replicated): feat_aug = h @ [fc_w_perm | A_l] written to a DRAM
    table G (bf16 rows, feat stored (f,h)-major + el); per-core er pair table
    ERL2n (row r: [er_{r-1} | er_r], leading pad row).
  - Phase B: per 128-node window (processed in pairs of windows), gather edge
    rows via InstDMAGatherAnt (int16 idx; lo/hi table split for the 32768
    index limit; one gather per section; 4 SWDGE queues). er per edge comes
    from the pair-neighbor table (one 16B row covers 2 edge slots; a static
    mask selects which half). w = max(exp(e), exp(0.2 e)) == exp(leaky_relu).
    One-hot selection rows built per chunk with a 4x-mode tensor_scalar
    (is_equal against an iota row); messages + softmax denominators
    accumulate into PSUM with TensorE matmuls (single-pass softmax).
  - Phase C (per window): rst = msg/z; out linear (PE transpose + matmul);
    stage x into xall.
  - Phase D (batched): folded bias (gat_bias @ out_w + out_b) + layernorm
    over all windows at once; single DMA out.
"""
import os
import numpy as np

import concourse.bass as bass
import concourse.bacc as bacc
import concourse.mybir as mybir
import concourse.tile as tile
import concourse.bass_utils as bu
from concourse.bass_utils import run_bass_kernel_spmd
from concourse.masks import make_identity
from concourse.tile_rust import add_dep_helper
from concourse import ap_utils
from concourse._compat import exact_div

# ---------------- constants ----------------
N, E, F, H = 50000, 800000, 64, 4
HF = H * F
NCORES = 8
NPC = N // NCORES            # 6250
P = 128
NWIN = (NPC + P - 1) // P    # 49
LO = 32768                   # int16 index split point
GROWS = 50176                # 49*1024, padded node count for phase A supertiles
GSTRIDE = 384                # bf16 elems per G row (768B, mult of 256B)
GROW_USED = 260              # feat 256 (f,h)-major | el 4
ERROWS = 6272                # 49*128
ER2ROWS = ERROWS + 1         # leading pad row
ERSTRIDE = 128               # bf16 elems per er row (256B)
LN_EPS = 1e-5
NEG = 0.2
F32 = mybir.dt.float32
BF16 = mybir.dt.bfloat16
I16 = mybir.dt.int16
I8 = mybir.dt.int8

# ---------------- walrus DGE patch (vector-indirect DMA support) ------------
_DGE_FLAG = "--dge-levels=vector_dynamic_offsets,dst_reduce"
_orig_bvo = bu.bir_verify_and_optimise

def _patched_bvo(tmpdir, inp="bir.json", outp="file.neff", arch=None, *, dve_root=None):
    orig_run = bu.run_command
    def run2(cmd, **kw):
        cmd = list(cmd)
        cmd.insert(1, _DGE_FLAG)
        return orig_run(cmd, **kw)
    bu.run_command = run2
    try:
        return _orig_bvo(tmpdir, inp, outp, arch, dve_root=dve_root)
    finally:
        bu.run_command = orig_run

bu.bir_verify_and_optimise = _patched_bvo


def dma_gather_relaxed(eng, out_ap, in_ap, idxs_ap, num_idxs_reg, num_idxs, elem_size,
                       elem_step, queue_num=0, single_packet=False):
    """nc.gpsimd.dma_gather minus the elem_size%256 assert (stride must still
    be a multiple of 256B; read length per row may be arbitrary)."""
    assert idxs_ap.dtype == I16
    assert in_ap.space == bass.MemorySpace.DRAM
    assert idxs_ap.space == bass.MemorySpace.SBUF
    assert out_ap.space == bass.MemorySpace.SBUF
    assert ap_utils.ap_is_contiguous(in_ap.ap[1:])
    assert ap_utils.ap_is_contiguous(out_ap.ap[1:])
    assert ap_utils.ap_is_contiguous(idxs_ap.ap[1:])
    assert in_ap.ap[-1][1] == out_ap.ap[-1][1] == elem_size
    assert out_ap.ap[0][1] * out_ap.ap[1][1] == num_idxs, (out_ap.ap, num_idxs)
    assert in_ap.ap[0][0] == elem_step
    stride_bytes = elem_step * mybir.dt.size(in_ap.dtype)
    stride_bytes_256 = exact_div(stride_bytes, 256)
    assert stride_bytes_256 < 256
    _in_ap = eng.lower_ap_dma(in_ap, for_custom_bir_dma=True)
    _idxs_ap = eng.lower_ap(idxs_ap)
    _out_ap = eng.lower_ap(out_ap)
    return eng.add_instruction(
        mybir.InstDMAGatherAnt(
            name=eng.bass.get_next_instruction_name(),
            ins=[*_in_ap, _idxs_ap, eng.lower_val_access(num_idxs_reg)],
            outs=[_out_ap],
            transpose=False,
            num_idxs=num_idxs,
            elem_size=elem_size,
            stride_bytes_256=stride_bytes_256,
            gen_mode=0,
            single_packet=single_packet,
            queue_num=queue_num,
            sbuf_tokens_per_rank=0,
            sbuf_free_dim_per_rank=0,
            sbuf_free_dim_pad_per_rank=0,
            sbuf_byte_offset=0,
        )
    )


def _groups():
    gs = [(w, w + 1) for w in range(0, NWIN - 1, 2)]
    if NWIN % 2:
        gs.append((NWIN - 1,))
    return gs


def _geom(CLw, CHw):
    """Static per-group geometry shared by host and device.

    Per group: ordered section list [(win, 'lo'/'hi', ncols)], column base
    offsets, er-jcol counts (ceil(ncols/2) per section), running global
    offsets for srcq (per column) and erq (per jcol)."""
    groups = _groups()
    geoms = []
    colbase = 0
    jcolbase = 0
    for grp in groups:
        secs = []
        for wv in grp:
            secs.append((wv, 'lo', int(CLw[wv])))
        for wv in grp:
            secs.append((wv, 'hi', int(CHw[wv])))
        lo_ch = sum(s[2] for s in secs if s[1] == 'lo')
        GC = sum(s[2] for s in secs)
        jcols = [(nc_ + 1) // 2 for (_, _, nc_) in secs]
        geoms.append(dict(grp=grp, secs=secs, lo_ch=lo_ch, GC=GC,
                          jcols=jcols, njc=sum(jcols),
                          colbase=colbase, jcolbase=jcolbase))
        colbase += GC
        jcolbase += sum(jcols)
    return geoms, colbase, jcolbase


def build_program(CLw, CHw):
    CLw = list(CLw); CHw = list(CHw)
    geoms, TOTC, TOTJ = _geom(CLw, CHw)
    nc = bacc.Bacc("TRN2", target_bir_lowering=False, debug=False,
                   num_devices=NCORES, dynamic_dma_scratch_size=65536,
                   num_swdge_queues=4)

    hT_p = nc.declare_dram_parameter("hT", [F, GROWS], BF16, isOutput=False)
    hTo_p = nc.declare_dram_parameter("hTo", [F, ERROWS], BF16, isOutput=False)
    Waug_p = nc.declare_dram_parameter("Waug", [F, 264], BF16, isOutput=False)
    outw_p = nc.declare_dram_parameter("outw", [HF, F], F32, isOutput=False)  # (f,h)-permuted
    vecs_p = nc.declare_dram_parameter("vecs", [P, 3, F], F32, isOutput=False)
    srcq_p = nc.declare_dram_parameter("srcq", [P, TOTC * 8], I16, isOutput=False)
    erq_p = nc.declare_dram_parameter("erq", [P, TOTJ * 8], I16, isOutput=False)
    mask_p = nc.declare_dram_parameter("mask", [P, TOTC, 4], I8, isOutput=False)
    dstf_p = nc.declare_dram_parameter("dstf", [P, TOTC], F32, isOutput=False)
    out_p = nc.declare_dram_parameter("out", [NWIN * P, F], F32, isOutput=True)

    G = nc.dram_tensor("G", [GROWS, GSTRIDE], BF16)
    ERL2 = nc.dram_tensor("ERL2", [ER2ROWS, ERSTRIDE], BF16)

    qctr = [0]
    def nextq():
        q = qctr[0] % 4
        qctr[0] += 1
        return q

    with tile.TileContext(nc) as tc:
        with tc.tile_pool(name="const", bufs=1) as cp:
            iota_t = cp.tile([P, P], BF16)
            nc.gpsimd.iota(iota_t[:], pattern=[[1, P]], base=0,
                           channel_multiplier=0, allow_small_or_imprecise_dtypes=True)
            ident = cp.tile([P, P], F32)
            make_identity(nc, ident[:])
            Waug_t = cp.tile([F, 264], BF16)
            nc.sync.dma_start(out=Waug_t[:], in_=Waug_p[:])
            outw_t = cp.tile([P, 2, F], F32)
            nc.sync.dma_start(out=outw_t[:], in_=outw_p[:].rearrange("(k p) f -> p k f", p=P))
            vecs_t = cp.tile([P, 3, F], F32)
            nc.sync.dma_start(out=vecs_t[:], in_=vecs_p[:])
            srcq_t = cp.tile([P, TOTC * 8], I16)
            nc.sync.dma_start(out=srcq_t[:], in_=srcq_p[:])
            erq_t = cp.tile([P, TOTJ * 8], I16)
            nc.sync.dma_start(out=erq_t[:], in_=erq_p[:])
            mask_t = cp.tile([P, TOTC, 4], I8)
            nc.sync.dma_start(out=mask_t[:], in_=mask_p[:])
            dstf_t = cp.tile([P, TOTC], F32)
            nc.sync.dma_start(out=dstf_t[:], in_=dstf_p[:])
            xall = cp.tile([P, NWIN, F], F32)

            # cached num_idxs registers
            _regs = {}
            def nreg(v):
                if v not in _regs:
                    _regs[v] = nc.gpsimd.to_reg(v)
                return _regs[v]

            lo_writes, er_writes, hi_writes = [], [], []
            # B-phase SBUF pools opened FIRST so their addresses don't overlap
            # phase-A staging (avoids false WAR deps stalling the first gathers)
            pools_b = [tc.tile_pool(name="phbl", bufs=3), tc.tile_pool(name="phb", bufs=2),
                       tc.tile_pool(name="phsel", bufs=2), tc.tile_pool(name="phc", bufs=4)]
            pbl, pb, psel, pc = [p.__enter__() for p in pools_b]
            # ---------------- phase A: G table (lo rows first, then er, then hi) ----
            # Waug feat cols are host-permuted to (f,h)-major, so copies are straight.
            with tc.tile_pool(name="pha", bufs=3) as pa, \
                 tc.tile_pool(name="phaps", bufs=3, space="PSUM") as pap, \
                 tc.tile_pool(name="phaps2", bufs=2, space="PSUM") as pap2:
                def do_supertile(st, sink):
                    ht = pa.tile([F, 1024], BF16, tag="ht")
                    nc.sync.dma_start(out=ht[:], in_=hT_p[:, st * 1024:(st + 1) * 1024])
                    stg = pa.tile([P, 8, GROW_USED], BF16, tag="stg")
                    elb = pap2.tile([P, 8, 4], F32, tag="psE")
                    for pr in range(4):
                        psF = pap.tile([P, 2, 256], F32, tag="psF")
                        for k in range(2):
                            t = 2 * pr + k
                            nc.tensor.matmul(out=psF[:, k, :], lhsT=ht[:, t * P:(t + 1) * P],
                                             rhs=Waug_t[:, 0:256], start=True, stop=True)
                            nc.tensor.matmul(out=elb[:, t, :], lhsT=ht[:, t * P:(t + 1) * P],
                                             rhs=Waug_t[:, 256:260], start=True, stop=True)
                        if pr == 3:
                            nc.vector.tensor_copy(out=stg[:, 2 * pr:2 * pr + 2, 0:256], in_=psF[:])
                        else:
                            nc.scalar.copy(out=stg[:, 2 * pr:2 * pr + 2, 0:256], in_=psF[:])
                    nc.scalar.copy(out=stg[:, :, 256:260], in_=elb[:])
                    wi = nc.sync.dma_start(
                        out=G[st * 1024:(st + 1) * 1024, 0:GROW_USED].rearrange(
                            "(s p) c -> p s c", p=P),
                        in_=stg[:])
                    sink.append(wi)
                # ---------------- phase A2 first: er pair table (own nodes) ---------
                for sg in range(7):
                    hb = pa.tile([F, 896], BF16, tag="hb")
                    nc.sync.dma_start(out=hb[:], in_=hTo_p[:, sg * 896:(sg + 1) * 896])
                    stg2 = pa.tile([P, 7, 4], BF16, tag="stg2")
                    for t in range(7):
                        ps2 = pap2.tile([P, 4], F32, tag="psA2")
                        nc.tensor.matmul(out=ps2[:], lhsT=hb[:, t * P:(t + 1) * P],
                                         rhs=Waug_t[:, 260:264], start=True, stop=True)
                        nc.scalar.copy(out=stg2[:, t, :], in_=ps2[:])
                    # row r of ERL2 = [er_{r-1} | er_r]; leading pad row 0
                    wi = nc.sync.dma_start(
                        out=ERL2[1 + sg * 896:1 + (sg + 1) * 896, 0:4].rearrange(
                            "(s p) c -> p s c", p=P),
                        in_=stg2[:])
                    er_writes.append(wi)
                    wi = nc.sync.dma_start(
                        out=ERL2[sg * 896:(sg + 1) * 896, 4:8].rearrange(
                            "(s p) c -> p s c", p=P),
                        in_=stg2[:])
                    er_writes.append(wi)
                join_er = nc.gpsimd.nop(nofuse=True)
                for wi in er_writes:
                    add_dep_helper(join_er.ins, wi.ins, reason="join_er waits on er writes")
                for st in range(LO // 1024):
                    do_supertile(st, lo_writes)
                join_lo = nc.gpsimd.nop(nofuse=True)
                for wi in lo_writes:
                    add_dep_helper(join_lo.ins, wi.ins, reason="join_lo waits on lo writes")
                for st in range(LO // 1024, GROWS // 1024):
                    do_supertile(st, hi_writes)

            join_hi_box = []

            def get_join_hi():
                if not join_hi_box:
                    jh = nc.gpsimd.nop(nofuse=True)
                    for wi in hi_writes:
                        add_dep_helper(jh.ins, wi.ins, reason="join_hi waits on hi writes")
                    join_hi_box.append(jh)
                return join_hi_box[0]

            # ---------------- phase B + C ----------------
            with tc.tile_pool(name="phbps", bufs=4, space="PSUM") as pwp, \
                 tc.tile_pool(name="phcps", bufs=2, space="PSUM") as pcp:
                PRE = 2  # groups whose lo/er gathers are prefetched ahead
                Xts, ERts = {}, {}

                def emit_lo_er(gi):
                    gm = geoms[gi]
                    lo_ch, njc = gm['lo_ch'], gm['njc']
                    b8 = gm['colbase'] * 8
                    j8 = gm['jcolbase'] * 8
                    Xlo = pbl.tile([P, lo_ch, GROW_USED], BF16, tag="Xlo")
                    ER2t = pbl.tile([P, njc, 8], BF16, tag="ER2t")
                    Xts[gi], ERts[gi] = Xlo, ER2t
                    g = dma_gather_relaxed(
                        nc.gpsimd, out_ap=Xlo[:], in_ap=G[0:LO, 0:GROW_USED],
                        idxs_ap=srcq_t[:, b8:b8 + lo_ch * 8],
                        num_idxs_reg=nreg(lo_ch * P),
                        num_idxs=lo_ch * P, elem_size=GROW_USED,
                        elem_step=GSTRIDE, queue_num=nextq())
                    add_dep_helper(g.ins, join_lo.ins, reason="gather after tables")
                    g = dma_gather_relaxed(
                        nc.gpsimd, out_ap=ER2t[:], in_ap=ERL2[:, 0:8],
                        idxs_ap=erq_t[:, j8:j8 + njc * 8],
                        num_idxs_reg=nreg(njc * P),
                        num_idxs=njc * P, elem_size=8,
                        elem_step=ERSTRIDE, queue_num=nextq())
                    add_dep_helper(g.ins, join_er.ins, reason="gather after tables")

                def complete(gi):
                    gm = geoms[gi]
                    grp, secs = gm['grp'], gm['secs']
                    W = len(grp)
                    lo_ch, GC, njc = gm['lo_ch'], gm['GC'], gm['njc']
                    gbase = gm['colbase']
                    b8 = gbase * 8
                    Xlo, ER2t = Xts.pop(gi), ERts.pop(gi)
                    hi_ch = GC - lo_ch
                    Xhi = pb.tile([P, hi_ch, GROW_USED], BF16, tag="Xhi")
                    g = dma_gather_relaxed(
                        nc.gpsimd, out_ap=Xhi[:], in_ap=G[LO:GROWS, 0:GROW_USED],
                        idxs_ap=srcq_t[:, b8 + lo_ch * 8:b8 + GC * 8],
                        num_idxs_reg=nreg(hi_ch * P),
                        num_idxs=hi_ch * P, elem_size=GROW_USED,
                        elem_step=GSTRIDE, queue_num=nextq())
                    add_dep_helper(g.ins, get_join_hi().ins, reason="gather after tables")
                    # er per edge slot from pair-neighbor rows (static mask mux)
                    ere = pb.tile([P, GC, 4], BF16, tag="ere")
                    c0 = 0
                    j0 = 0
                    for (wv, kind, ncols) in secs:
                        if ncols == 0:
                            continue
                        nb = ncols // 2
                        tail = ncols % 2
                        if nb:
                            nc.vector.select(
                                out=ere[:, c0:c0 + 2 * nb, :].rearrange(
                                    "p (k b) h -> p k b h", b=2),
                                mask=mask_t[:, gbase + c0:gbase + c0 + 2 * nb, :].rearrange(
                                    "p (k b) h -> p k b h", b=2),
                                on_true=ER2t[:, j0:j0 + nb, None, 4:8].broadcast_to(
                                    [P, nb, 2, 4]),
                                on_false=ER2t[:, j0:j0 + nb, None, 0:4].broadcast_to(
                                    [P, nb, 2, 4]))
                        if tail:
                            nc.vector.tensor_copy(
                                out=ere[:, c0 + 2 * nb:c0 + ncols, :],
                                in_=ER2t[:, j0 + nb:j0 + nb + 1, 0:4])
                        c0 += ncols
                        j0 += nb + tail
                    # per-half w pipeline so lo-chunk matmuls start before hi gathers land
                    wb = pb.tile([P, GC, 4], BF16, tag="wb")
                    sel = psel.tile([P, GC, P], BF16, tag="sel")
                    ew = pb.tile([P, GC, 4], BF16, tag="ew")
                    w1 = pb.tile([P, GC, 4], BF16, tag="w1")
                    w2 = pb.tile([P, GC, 4], BF16, tag="w2")
                    for (h0, h1, Xh) in ((0, lo_ch, Xlo), (lo_ch, GC, Xhi)):
                        hs = slice(h0, h1)
                        xs = slice(0, h1 - h0)
                        nh = h1 - h0
                        nc.vector.tensor_tensor(out=ew[:, hs, :], in0=Xh[:, xs, 256:260],
                                                in1=ere[:, hs, :], op=mybir.AluOpType.add)
                        nc.scalar.activation(out=w1[:, hs, :], in_=ew[:, hs, :],
                                             func=mybir.ActivationFunctionType.Exp)
                        nc.scalar.activation(out=w2[:, hs, :], in_=ew[:, hs, :],
                                             func=mybir.ActivationFunctionType.Exp, scale=NEG)
                        nc.vector.tensor_tensor(out=wb[:, hs, :], in0=w1[:, hs, :],
                                                in1=w2[:, hs, :], op=mybir.AluOpType.max)
                        # fold w into features in place ((f,h)-packed broadcast)
                        nc.vector.tensor_tensor(
                            out=Xh[:, xs, 0:256].rearrange("p j (f h) -> p j f h", f=F),
                            in0=Xh[:, xs, 0:256].rearrange("p j (f h) -> p j f h", f=F),
                            in1=wb[:, hs, None, :].broadcast_to([P, nh, F, H]),
                            op=mybir.AluOpType.mult)
                        nc.vector.tensor_copy(out=Xh[:, xs, 256:260], in_=wb[:, hs, :])
                    # chunk -> window-slot ownership per section order
                    own = []
                    for (wv, kind, ncols) in secs:
                        own += [grp.index(wv)] * ncols
                    first = {i: own.index(i) for i in range(W)}
                    last = {i: GC - 1 - own[::-1].index(i) for i in range(W)}
                    psws = []
                    for _pi in range(W):
                        psw_t = pwp.tile([P, GROW_USED], F32, tag="psw")
                        psws.append(psw_t)
                    for c in range(GC):
                        o = own[c]
                        rhs = Xlo[:, c, :] if c < lo_ch else Xhi[:, c - lo_ch, :]
                        nc.vector.tensor_scalar(out=sel[:, c, :], in0=iota_t[:],
                                                scalar1=dstf_t[:, gbase + c:gbase + c + 1],
                                                scalar2=None,
                                                op0=mybir.AluOpType.is_equal)
                        nc.tensor.matmul(out=psws[o][:], lhsT=sel[:, c, :], rhs=rhs,
                                         start=(c == first[o]), stop=(c == last[o]),
                                         skip_group_check=True)
                    # ---- phase C (per window in group): normalize, out-linear, stage x ----
                    for wi_, wv in enumerate(grp):
                        psw = psws[wi_]
                        zs = pc.tile([P, 4], F32, tag="zs")
                        nc.vector.tensor_scalar(out=zs[:], in0=psw[:, 256:260],
                                                scalar1=1e-30, scalar2=None,
                                                op0=mybir.AluOpType.max)
                        zr = pc.tile([P, 4], F32, tag="zr")
                        nc.vector.reciprocal_approx_fast(out=zr[:], in_=zs[:])
                        rstn = pc.tile([P, HF], F32, tag="rstn")
                        nc.vector.tensor_tensor(out=rstn[:].rearrange("p (f h) -> p f h", f=F),
                                                in0=psw[:, 0:256].rearrange("p (f h) -> p f h", f=F),
                                                in1=zr[:, None, :].broadcast_to([P, F, H]),
                                                op=mybir.AluOpType.mult)
                        psx = pcp.tile([P, F], F32, tag="psx")
                        for half in range(2):
                            pst = pcp.tile([P, P], F32, tag="pst")
                            nc.tensor.transpose(out=pst[:], in_=rstn[:, half * P:(half + 1) * P],
                                                identity=ident[:])
                            rT = pc.tile([P, P], F32, tag="rT")
                            nc.scalar.copy(out=rT[:], in_=pst[:])
                            nc.tensor.matmul(out=psx[:], lhsT=rT[:], rhs=outw_t[:, half, :],
                                             start=(half == 0), stop=(half == 1))
                        nc.scalar.copy(out=xall[:, wv, :], in_=psx[:])

                for gi in range(len(geoms) + PRE):
                    if gi < len(geoms):
                        emit_lo_er(gi)
                    if gi >= PRE:
                        complete(gi - PRE)

                # ---- phase D: batched bias + layernorm over all windows ----
                with tc.tile_pool(name="phd", bufs=1) as pd:
                    nc.vector.tensor_tensor(
                        out=xall[:], in0=xall[:],
                        in1=vecs_t[:, 0:1, :].broadcast_to([P, NWIN, F]),
                        op=mybir.AluOpType.add)
                    s1 = pd.tile([P, NWIN], F32, tag="s1")
                    nc.vector.tensor_reduce(out=s1[:], in_=xall[:],
                                            axis=mybir.AxisListType.X,
                                            op=mybir.AluOpType.add)
                    negmu = pd.tile([P, NWIN], F32, tag="negmu")
                    nc.vector.tensor_scalar(out=negmu[:], in0=s1[:], scalar1=-1.0 / F,
                                            scalar2=None, op0=mybir.AluOpType.mult)
                    nc.vector.tensor_tensor(
                        out=xall[:], in0=xall[:],
                        in1=negmu[:, :, None].broadcast_to([P, NWIN, F]),
                        op=mybir.AluOpType.add)
                    sq = pd.tile([P, NWIN, F], F32, tag="sq")
                    nc.vector.tensor_tensor(out=sq[:], in0=xall[:], in1=xall[:],
                                            op=mybir.AluOpType.mult)
                    ss = pd.tile([P, NWIN], F32, tag="ss")
                    nc.vector.tensor_reduce(out=ss[:], in_=sq[:],
                                            axis=mybir.AxisListType.X,
                                            op=mybir.AluOpType.add)
                    v = pd.tile([P, NWIN], F32, tag="v")
                    nc.vector.tensor_scalar(out=v[:], in0=ss[:], scalar1=1.0 / F,
                                            scalar2=LN_EPS, op0=mybir.AluOpType.mult,
                                            op1=mybir.AluOpType.add)
                    sv = pd.tile([P, NWIN], F32, tag="sv")
                    nc.scalar.activation(out=sv[:], in_=v[:],
                                         func=mybir.ActivationFunctionType.Sqrt)
                    rstd = pd.tile([P, NWIN], F32, tag="rstd")
                    nc.vector.reciprocal_approx_fast(out=rstd[:], in_=sv[:])
                    nc.vector.tensor_tensor(
                        out=xall[:], in0=xall[:],
                        in1=rstd[:, :, None].broadcast_to([P, NWIN, F]),
                        op=mybir.AluOpType.mult)
                    nc.vector.tensor_tensor(
                        out=xall[:], in0=xall[:],
                        in1=vecs_t[:, 1:2, :].broadcast_to([P, NWIN, F]),
                        op=mybir.AluOpType.mult)
                    nc.vector.tensor_tensor(
                        out=xall[:], in0=xall[:],
                        in1=vecs_t[:, 2:3, :].broadcast_to([P, NWIN, F]),
                        op=mybir.AluOpType.add)
                    nc.sync.dma_start(
                        out=out_p[:].rearrange("(w p) f -> p w f", p=P), in_=xall[:])
            for p in reversed(pools_b):
                p.__exit__(None, None, None)

    nc.compile()
    return nc


# ---------------- host side ----------------
def host_prep(h, src, dst, fc_w, attn_l, attn_r, gat_bias, out_w, out_b, ln_g, ln_b):
    h = np.ascontiguousarray(np.asarray(h, np.float32))
    src = np.asarray(src, np.int64)
    dst = np.asarray(dst, np.int64)
    fc_w = np.asarray(fc_w, np.float32)
    attn_l = np.asarray(attn_l, np.float32)
    attn_r = np.asarray(attn_r, np.float32)
    gat_bias = np.asarray(gat_bias, np.float32)
    out_w = np.asarray(out_w, np.float32)
    out_b = np.asarray(out_b, np.float32)
    ln_g = np.asarray(ln_g, np.float32)
    ln_b = np.asarray(ln_b, np.float32)

    A_l = np.einsum('khf,hf->kh', fc_w.reshape(F, H, F), attn_l).astype(np.float32)
    A_r = np.einsum('khf,hf->kh', fc_w.reshape(F, H, F), attn_r).astype(np.float32)
    # fc cols permuted to (f,h)-major so phase A writes G rows without a permute
    perm = (np.arange(F)[:, None] + np.arange(H)[None, :] * F).reshape(-1)
    Waug = np.ascontiguousarray(np.concatenate([fc_w[:, perm], A_l, A_r], axis=1))  # [64, 264]
    bias2 = (gat_bias @ out_w + out_b).astype(np.float32)                  # [64]
    # out_w permuted to (f,h)-major rows to match the G feat layout
    outw_perm = np.ascontiguousarray(
        out_w.reshape(H, F, F).transpose(1, 0, 2).reshape(HF, F))

    import ml_dtypes
    hT = np.zeros((F, GROWS), ml_dtypes.bfloat16)
    hT[:, :N] = h.T.astype(ml_dtypes.bfloat16)
    hTo = np.zeros((NCORES, F, ERROWS), ml_dtypes.bfloat16)
    for c in range(NCORES):
        hTo[c, :, :NPC] = h[c * NPC:(c + 1) * NPC].T.astype(ml_dtypes.bfloat16)

    vecs = np.zeros((P, 3, F), np.float32)
    vecs[:, 0, :] = bias2
    vecs[:, 1, :] = ln_g
    vecs[:, 2, :] = ln_b

    # sort edges by dst
    order = np.argsort(dst, kind='stable')
    ssrc = src[order]
    sdst = dst[order]
    core_of = sdst // NPC
    loc = sdst - core_of * NPC
    win = loc // P
    gw = core_of * NWIN + win
    counts = np.bincount(gw, minlength=NCORES * NWIN)
    starts = np.zeros(NCORES * NWIN + 1, np.int64)
    np.cumsum(counts, out=starts[1:])
    lomask = ssrc < LO

    def pack_section(e_src, e_loc, wv):
        """Pack a section's edges into pair slots. Pair t = (p=t%P, blk=t//P)
        covers cols (2blk, 2blk+1); consecutive dst-sorted edges pair up when
        the 2nd edge's local row is r0 or r0+1 (one 16B table row covers both).
        Returns (npairs, lists-of (pair_t, b, src, dloc, mk, eridx))."""
        ne = len(e_src)
        slots = []
        i, t = 0, 0
        while i < ne:
            r0 = int(e_loc[i])
            slots.append((t, 0, int(e_src[i]), int(e_loc[i]) - wv * P, 0.0, r0 + 1))
            i += 1
            if i < ne:
                r1 = int(e_loc[i])
                if r1 == r0 or r1 == r0 + 1:
                    slots.append((t, 1, int(e_src[i]), r1 - wv * P,
                                  float(r1 - r0), r0 + 1))
                    i += 1
            t += 1
        return t, slots

    # first pass: pack and derive static per-window chunk counts (max / cores)
    packed = {}
    CLw = np.ones(NWIN, np.int64)
    CHw = np.ones(NWIN, np.int64)
    for c in range(NCORES):
        for w in range(NWIN):
            g = c * NWIN + w
            sl = slice(starts[g], starts[g + 1])
            m = lomask[sl]
            s_src = ssrc[sl]
            s_loc = loc[sl]
            for kind, es, el_ in (('lo', s_src[m], s_loc[m]),
                                  ('hi', s_src[~m] - LO, s_loc[~m])):
                npairs, slots = pack_section(es, el_, w)
                packed[(c, w, kind)] = (npairs, slots)
                need = 1
                for (t, b, _s, _d, _m, _e) in slots:
                    need = max(need, 2 * (t // P) + b + 1)
                if kind == 'lo':
                    CLw[w] = max(CLw[w], need)
                else:
                    CHw[w] = max(CHw[w], need)

    geoms, TOTC, TOTJ = _geom(CLw, CHw)

    srcq = np.zeros((NCORES, P, TOTC * 8), np.int16)
    erq = np.ones((NCORES, P, TOTJ * 8), np.int16)
    maskq = np.zeros((NCORES, P, TOTC, 4), np.int8)
    dstf = np.full((NCORES, P, TOTC), 200.0, np.float32)

    def tile16(a):
        # idx layout for the gather: [16, n*8] wrapped, replicated to 128 rows
        ncols = a.shape[0]
        return np.tile(a.reshape(ncols * 8, 16).T, (8, 1))

    for c in range(NCORES):
        for gm in geoms:
            colbase, jcolbase = gm['colbase'], gm['jcolbase']
            c0, j0 = 0, 0
            for (wv, kind, ncols) in gm['secs']:
                nb = ncols // 2
                tail = ncols % 2
                npairs, slots = packed[(c, wv, kind)]
                sq = np.zeros((ncols, P), np.int16)
                df = np.full((ncols, P), 200.0, np.float32)
                mk = np.zeros((ncols, P), np.float32)
                ej = np.ones((nb + tail, P), np.int16)
                for (t, b, s_, dl, m_, er_) in slots:
                    p_ = t % P
                    blk = t // P
                    col = 2 * blk + b
                    assert col < ncols, (c, wv, kind, col, ncols)
                    sq[col, p_] = s_
                    df[col, p_] = dl
                    mk[col, p_] = m_
                    jc = blk if (2 * blk + 1 < ncols) else nb  # tail jcol
                    if 2 * blk + 1 >= ncols:
                        assert b == 0
                    ej[jc, p_] = er_
                cb = colbase + c0
                jb = jcolbase + j0
                srcq[c][:, cb * 8:(cb + ncols) * 8] = tile16(sq)
                erq[c][:, jb * 8:(jb + nb + tail) * 8] = tile16(ej)
                dstf[c][:, cb:cb + ncols] = df.T
                maskq[c][:, cb:cb + ncols, :] = mk.T[:, :, None]
                c0 += ncols
                j0 += nb + tail

    small = dict(Waug=Waug.astype(ml_dtypes.bfloat16), outw=outw_perm, vecs=vecs)
    return hT, hTo, srcq, erq, maskq, dstf, small, tuple(CLw), tuple(CHw)


_prog_cache = {}

def kernel(**inputs):
    hT, hTo, srcq, erq, maskq, dstf, small, CLw, CHw = host_prep(**inputs)
    key = (CLw, CHw)
    if key not in _prog_cache:
        _prog_cache[key] = build_program(CLw, CHw)
    nc = _prog_cache[key]
    in_maps = []
    for c in range(NCORES):
        in_maps.append({
            "hT": hT, "hTo": hTo[c], "Waug": small["Waug"], "outw": small["outw"],
            "vecs": small["vecs"], "srcq": srcq[c], "erq": erq[c],
            "mask": maskq[c], "dstf": dstf[c],
        })
    res = run_bass_kernel_spmd(nc, in_maps, list(range(NCORES)))
    out = np.concatenate([np.asarray(res.results[c]["out"])[:NPC] for c in range(NCORES)], axis=0)
    return out


# revision 14
# speedup vs baseline: 1.0777x; 1.0145x over previous
"""GAT message-passing kernel for Trainium2 (8 NeuronCores, SPMD).

Strategy (dst-partitioned, no collectives):
  - Sort edges by dst on host; core c owns dst nodes [c*6250, (c+1)*6250).
  - Phase A (replicated): feat_aug = h @ [fc_w_perm | A_l] written to a DRAM
    table G (bf16 rows, feat stored (f,h)-major + el); per-core er pair table
    ERL2n (row r: [er_{r-1} | er_r], leading pad row).
  - Phase B: per 128-node window (processed in pairs of windows), gather edge
    rows via InstDMAGatherAnt (int16 idx; lo/hi table split for the 32768
    index limit; one gather per section; 4 SWDGE queues). er per edge comes
    from the pair-neighbor table (one 16B row covers 2 edge slots; a static
    mask selects which half). w = max(exp(e), exp(0.2 e)) == exp(leaky_relu).
    One-hot selection rows built per chunk with a 4x-mode tensor_scalar
    (is_equal against an iota row); messages + softmax denominators
    accumulate into PSUM with TensorE matmuls (single-pass softmax).
  - Phase C (per window): rst = msg/z; out linear (PE transpose + matmul);
    stage x into xall.
  - Phase D (batched): folded bias (gat_bias @ out_w + out_b) + layernorm
    over all windows at once; single DMA out.
"""
import os
import numpy as np

import concourse.bass as bass
import concourse.bacc as bacc
import concourse.mybir as mybir
import concourse.tile as tile
import concourse.bass_utils as bu
from concourse.bass_utils import run_bass_kernel_spmd
from concourse.masks import make_identity
from concourse.tile_rust import add_dep_helper
from concourse import ap_utils
from concourse._compat import exact_div

# ---------------- constants ----------------
N, E, F, H = 50000, 800000, 64, 4
HF = H * F
NCORES = 8
NPC = N // NCORES            # 6250
P = 128
NWIN = (NPC + P - 1) // P    # 49
LO = 32768                   # int16 index split point
GROWS = 50176                # 49*1024, padded node count for phase A supertiles
GSTRIDE = 384                # bf16 elems per G row (768B, mult of 256B)
GROW_USED = 260              # feat 256 (f,h)-major | el 4
ERROWS = 6272                # 49*128
ER2ROWS = ERROWS + 1         # leading pad row
ERSTRIDE = 128               # bf16 elems per er row (256B)
LN_EPS = 1e-5
NEG = 0.2
F32 = mybir.dt.float32
BF16 = mybir.dt.bfloat16
I16 = mybir.dt.int16
I8 = mybir.dt.int8

# ---------------- walrus DGE patch (vector-indirect DMA support) ------------
_DGE_FLAG = "--dge-levels=vector_dynamic_offsets,dst_reduce"
_orig_bvo = bu.bir_verify_and_optimise

def _patched_bvo(tmpdir, inp="bir.json", outp="file.neff", arch=None, *, dve_root=None):
    orig_run = bu.run_command
    def run2(cmd, **kw):
        cmd = list(cmd)
        cmd.insert(1, _DGE_FLAG)
        return orig_run(cmd, **kw)
    bu.run_command = run2
    try:
        return _orig_bvo(tmpdir, inp, outp, arch, dve_root=dve_root)
    finally:
        bu.run_command = orig_run

bu.bir_verify_and_optimise = _patched_bvo


def dma_gather_relaxed(eng, out_ap, in_ap, idxs_ap, num_idxs_reg, num_idxs, elem_size,
                       elem_step, queue_num=0, single_packet=False):
    """nc.gpsimd.dma_gather minus the elem_size%256 assert (stride must still
    be a multiple of 256B; read length per row may be arbitrary)."""
    assert idxs_ap.dtype == I16
    assert in_ap.space == bass.MemorySpace.DRAM
    assert idxs_ap.space == bass.MemorySpace.SBUF
    assert out_ap.space == bass.MemorySpace.SBUF
    assert ap_utils.ap_is_contiguous(in_ap.ap[1:])
    assert ap_utils.ap_is_contiguous(out_ap.ap[1:])
    assert ap_utils.ap_is_contiguous(idxs_ap.ap[1:])
    assert in_ap.ap[-1][1] == out_ap.ap[-1][1] == elem_size
    assert out_ap.ap[0][1] * out_ap.ap[1][1] == num_idxs, (out_ap.ap, num_idxs)
    assert in_ap.ap[0][0] == elem_step
    stride_bytes = elem_step * mybir.dt.size(in_ap.dtype)
    stride_bytes_256 = exact_div(stride_bytes, 256)
    assert stride_bytes_256 < 256
    _in_ap = eng.lower_ap_dma(in_ap, for_custom_bir_dma=True)
    _idxs_ap = eng.lower_ap(idxs_ap)
    _out_ap = eng.lower_ap(out_ap)
    return eng.add_instruction(
        mybir.InstDMAGatherAnt(
            name=eng.bass.get_next_instruction_name(),
            ins=[*_in_ap, _idxs_ap, eng.lower_val_access(num_idxs_reg)],
            outs=[_out_ap],
            transpose=False,
            num_idxs=num_idxs,
            elem_size=elem_size,
            stride_bytes_256=stride_bytes_256,
            gen_mode=0,
            single_packet=single_packet,
            queue_num=queue_num,
            sbuf_tokens_per_rank=0,
            sbuf_free_dim_per_rank=0,
            sbuf_free_dim_pad_per_rank=0,
            sbuf_byte_offset=0,
        )
    )


def _groups():
    gs = [(w, w + 1) for w in range(0, NWIN - 1, 2)]
    if NWIN % 2:
        gs.append((NWIN - 1,))
    return gs


def _geom(CLw, CHw):
    """Static per-group geometry shared by host and device.

    Per group: ordered section list [(win, 'lo'/'hi', ncols)], column base
    offsets, er-jcol counts (ceil(ncols/2) per section), running global
    offsets for srcq (per column) and erq (per jcol)."""
    groups = _groups()
    geoms = []
    colbase = 0
    jcolbase = 0
    for grp in groups:
        secs = []
        for wv in grp:
            secs.append((wv, 'lo', int(CLw[wv])))
        for wv in grp:
            secs.append((wv, 'hi', int(CHw[wv])))
        lo_ch = sum(s[2] for s in secs if s[1] == 'lo')
        GC = sum(s[2] for s in secs)
        jcols = [(nc_ + 1) // 2 for (_, _, nc_) in secs]
        geoms.append(dict(grp=grp, secs=secs, lo_ch=lo_ch, GC=GC,
                          jcols=jcols, njc=sum(jcols),
                          colbase=colbase, jcolbase=jcolbase))
        colbase += GC
        jcolbase += sum(jcols)
    return geoms, colbase, jcolbase


def build_program(CLw, CHw):
    CLw = list(CLw); CHw = list(CHw)
    geoms, TOTC, TOTJ = _geom(CLw, CHw)
    nc = bacc.Bacc("TRN2", target_bir_lowering=False, debug=False,
                   num_devices=NCORES, dynamic_dma_scratch_size=65536,
                   num_swdge_queues=4)

    hT_p = nc.declare_dram_parameter("hT", [F, GROWS], BF16, isOutput=False)
    hTo_p = nc.declare_dram_parameter("hTo", [F, ERROWS], BF16, isOutput=False)
    Waug_p = nc.declare_dram_parameter("Waug", [F, 264], BF16, isOutput=False)
    outw_p = nc.declare_dram_parameter("outw", [HF, F], F32, isOutput=False)  # (f,h)-permuted
    vecs_p = nc.declare_dram_parameter("vecs", [P, 3, F], F32, isOutput=False)
    srcq_p = nc.declare_dram_parameter("srcq", [P, TOTC * 8], I16, isOutput=False)
    erq_p = nc.declare_dram_parameter("erq", [P, TOTJ * 8], I16, isOutput=False)
    mask_p = nc.declare_dram_parameter("mask", [P, TOTC, 4], I8, isOutput=False)
    dstf_p = nc.declare_dram_parameter("dstf", [P, TOTC], F32, isOutput=False)
    out_p = nc.declare_dram_parameter("out", [NWIN * P, F], F32, isOutput=True)

    G = nc.dram_tensor("G", [GROWS, GSTRIDE], BF16)
    ERL2 = nc.dram_tensor("ERL2", [ER2ROWS, ERSTRIDE], BF16)

    qctr = [0]
    def nextq():
        q = qctr[0] % 4
        qctr[0] += 1
        return q

    with tile.TileContext(nc) as tc:
        with tc.tile_pool(name="const", bufs=1) as cp:
            iota_t = cp.tile([P, P], BF16)
            nc.gpsimd.iota(iota_t[:], pattern=[[1, P]], base=0,
                           channel_multiplier=0, allow_small_or_imprecise_dtypes=True)
            ident = cp.tile([P, P], F32)
            make_identity(nc, ident[:])
            Waug_t = cp.tile([F, 264], BF16)
            nc.sync.dma_start(out=Waug_t[:], in_=Waug_p[:])
            outw_t = cp.tile([P, 2, F], F32)
            nc.sync.dma_start(out=outw_t[:], in_=outw_p[:].rearrange("(k p) f -> p k f", p=P))
            vecs_t = cp.tile([P, 3, F], F32)
            nc.sync.dma_start(out=vecs_t[:], in_=vecs_p[:])
            srcq_t = cp.tile([P, TOTC * 8], I16)
            nc.sync.dma_start(out=srcq_t[:], in_=srcq_p[:])
            erq_t = cp.tile([P, TOTJ * 8], I16)
            nc.sync.dma_start(out=erq_t[:], in_=erq_p[:])
            mask_t = cp.tile([P, TOTC, 4], I8)
            nc.sync.dma_start(out=mask_t[:], in_=mask_p[:])
            dstf_t = cp.tile([P, TOTC], F32)
            nc.sync.dma_start(out=dstf_t[:], in_=dstf_p[:])
            xall = cp.tile([P, NWIN, F], F32)

            # cached num_idxs registers
            _regs = {}
            def nreg(v):
                if v not in _regs:
                    _regs[v] = nc.gpsimd.to_reg(v)
                return _regs[v]

            lo_writes, er_writes, hi_writes = [], [], []
            # B-phase SBUF pools opened FIRST so their addresses don't overlap
            # phase-A staging (avoids false WAR deps stalling the first gathers)
            pools_b = [tc.tile_pool(name="phbl", bufs=3), tc.tile_pool(name="phb", bufs=2),
                       tc.tile_pool(name="phsel", bufs=2), tc.tile_pool(name="phc", bufs=4)]
            pbl, pb, psel, pc = [p.__enter__() for p in pools_b]
            # ---------------- phase A: G table (lo rows first, then er, then hi) ----
            # Waug feat cols are host-permuted to (f,h)-major, so copies are straight.
            with tc.tile_pool(name="pha", bufs=3) as pa, \
                 tc.tile_pool(name="phaps", bufs=3, space="PSUM") as pap, \
                 tc.tile_pool(name="phaps2", bufs=2, space="PSUM") as pap2:
                def do_supertile(st, sink):
                    ht = pa.tile([F, 1024], BF16, tag="ht")
                    nc.sync.dma_start(out=ht[:], in_=hT_p[:, st * 1024:(st + 1) * 1024])
                    stg = pa.tile([P, 8, GROW_USED], BF16, tag="stg")
                    elb = pap2.tile([P, 8, 4], F32, tag="psE")
                    for pr in range(4):
                        psF = pap.tile([P, 2, 256], F32, tag="psF")
                        for k in range(2):
                            t = 2 * pr + k
                            nc.tensor.matmul(out=psF[:, k, :], lhsT=ht[:, t * P:(t + 1) * P],
                                             rhs=Waug_t[:, 0:256], start=True, stop=True)
                            nc.tensor.matmul(out=elb[:, t, :], lhsT=ht[:, t * P:(t + 1) * P],
                                             rhs=Waug_t[:, 256:260], start=True, stop=True)
                        if pr % 2 == 0:
                            nc.vector.tensor_copy(out=stg[:, 2 * pr:2 * pr + 2, 0:256], in_=psF[:])
                        else:
                            nc.scalar.copy(out=stg[:, 2 * pr:2 * pr + 2, 0:256], in_=psF[:])
                    nc.scalar.copy(out=stg[:, :, 256:260], in_=elb[:])
                    wi = nc.sync.dma_start(
                        out=G[st * 1024:(st + 1) * 1024, 0:GROW_USED].rearrange(
                            "(s p) c -> p s c", p=P),
                        in_=stg[:])
                    sink.append(wi)
                for st in range(LO // 1024):
                    do_supertile(st, lo_writes)
                join_lo = nc.gpsimd.nop(nofuse=True)
                for wi in lo_writes:
                    add_dep_helper(join_lo.ins, wi.ins, reason="join_lo waits on lo writes")
                # ---------------- phase A2: er pair table (own nodes) ---------
                for sg in range(7):
                    hb = pa.tile([F, 896], BF16, tag="hb")
                    nc.sync.dma_start(out=hb[:], in_=hTo_p[:, sg * 896:(sg + 1) * 896])
                    stg2 = pa.tile([P, 7, 4], BF16, tag="stg2")
                    for t in range(7):
                        ps2 = pap2.tile([P, 4], F32, tag="psA2")
                        nc.tensor.matmul(out=ps2[:], lhsT=hb[:, t * P:(t + 1) * P],
                                         rhs=Waug_t[:, 260:264], start=True, stop=True)
                        nc.scalar.copy(out=stg2[:, t, :], in_=ps2[:])
                    # row r of ERL2 = [er_{r-1} | er_r]; leading pad row 0
                    wi = nc.sync.dma_start(
                        out=ERL2[1 + sg * 896:1 + (sg + 1) * 896, 0:4].rearrange(
                            "(s p) c -> p s c", p=P),
                        in_=stg2[:])
                    er_writes.append(wi)
                    wi = nc.sync.dma_start(
                        out=ERL2[sg * 896:(sg + 1) * 896, 4:8].rearrange(
                            "(s p) c -> p s c", p=P),
                        in_=stg2[:])
                    er_writes.append(wi)
                join_er = nc.gpsimd.nop(nofuse=True)
                for wi in er_writes:
                    add_dep_helper(join_er.ins, wi.ins, reason="join_er waits on er writes")
                for st in range(LO // 1024, GROWS // 1024):
                    do_supertile(st, hi_writes)

            join_hi_box = []

            def get_join_hi():
                if not join_hi_box:
                    jh = nc.gpsimd.nop(nofuse=True)
                    for wi in hi_writes:
                        add_dep_helper(jh.ins, wi.ins, reason="join_hi waits on hi writes")
                    join_hi_box.append(jh)
                return join_hi_box[0]

            # ---------------- phase B + C ----------------
            with tc.tile_pool(name="phbps", bufs=4, space="PSUM") as pwp, \
                 tc.tile_pool(name="phcps", bufs=2, space="PSUM") as pcp:
                PRE = 2  # groups whose lo/er gathers are prefetched ahead
                Xts, ERts = {}, {}

                def emit_lo_er(gi):
                    gm = geoms[gi]
                    lo_ch, njc = gm['lo_ch'], gm['njc']
                    b8 = gm['colbase'] * 8
                    j8 = gm['jcolbase'] * 8
                    Xlo = pbl.tile([P, lo_ch, GROW_USED], BF16, tag="Xlo")
                    ER2t = pbl.tile([P, njc, 8], BF16, tag="ER2t")
                    Xts[gi], ERts[gi] = Xlo, ER2t
                    g = dma_gather_relaxed(
                        nc.gpsimd, out_ap=Xlo[:], in_ap=G[0:LO, 0:GROW_USED],
                        idxs_ap=srcq_t[:, b8:b8 + lo_ch * 8],
                        num_idxs_reg=nreg(lo_ch * P),
                        num_idxs=lo_ch * P, elem_size=GROW_USED,
                        elem_step=GSTRIDE, queue_num=nextq())
                    add_dep_helper(g.ins, join_lo.ins, reason="gather after tables")
                    g = dma_gather_relaxed(
                        nc.gpsimd, out_ap=ER2t[:], in_ap=ERL2[:, 0:8],
                        idxs_ap=erq_t[:, j8:j8 + njc * 8],
                        num_idxs_reg=nreg(njc * P),
                        num_idxs=njc * P, elem_size=8,
                        elem_step=ERSTRIDE, queue_num=nextq())
                    add_dep_helper(g.ins, join_er.ins, reason="gather after tables")

                def complete(gi):
                    gm = geoms[gi]
                    grp, secs = gm['grp'], gm['secs']
                    W = len(grp)
                    lo_ch, GC, njc = gm['lo_ch'], gm['GC'], gm['njc']
                    gbase = gm['colbase']
                    b8 = gbase * 8
                    Xlo, ER2t = Xts.pop(gi), ERts.pop(gi)
                    hi_ch = GC - lo_ch
                    Xhi = pb.tile([P, hi_ch, GROW_USED], BF16, tag="Xhi")
                    g = dma_gather_relaxed(
                        nc.gpsimd, out_ap=Xhi[:], in_ap=G[LO:GROWS, 0:GROW_USED],
                        idxs_ap=srcq_t[:, b8 + lo_ch * 8:b8 + GC * 8],
                        num_idxs_reg=nreg(hi_ch * P),
                        num_idxs=hi_ch * P, elem_size=GROW_USED,
                        elem_step=GSTRIDE, queue_num=nextq())
                    add_dep_helper(g.ins, get_join_hi().ins, reason="gather after tables")
                    # er per edge slot from pair-neighbor rows (static mask mux)
                    ere = pb.tile([P, GC, 4], BF16, tag="ere")
                    c0 = 0
                    j0 = 0
                    for (wv, kind, ncols) in secs:
                        if ncols == 0:
                            continue
                        nb = ncols // 2
                        tail = ncols % 2
                        if nb:
                            nc.vector.select(
                                out=ere[:, c0:c0 + 2 * nb, :].rearrange(
                                    "p (k b) h -> p k b h", b=2),
                                mask=mask_t[:, gbase + c0:gbase + c0 + 2 * nb, :].rearrange(
                                    "p (k b) h -> p k b h", b=2),
                                on_true=ER2t[:, j0:j0 + nb, None, 4:8].broadcast_to(
                                    [P, nb, 2, 4]),
                                on_false=ER2t[:, j0:j0 + nb, None, 0:4].broadcast_to(
                                    [P, nb, 2, 4]))
                        if tail:
                            nc.vector.tensor_copy(
                                out=ere[:, c0 + 2 * nb:c0 + ncols, :],
                                in_=ER2t[:, j0 + nb:j0 + nb + 1, 0:4])
                        c0 += ncols
                        j0 += nb + tail
                    # per-half w pipeline so lo-chunk matmuls start before hi gathers land
                    wb = pb.tile([P, GC, 4], BF16, tag="wb")
                    sel = psel.tile([P, GC, P], BF16, tag="sel")
                    ew = pb.tile([P, GC, 4], BF16, tag="ew")
                    w1 = pb.tile([P, GC, 4], BF16, tag="w1")
                    w2 = pb.tile([P, GC, 4], BF16, tag="w2")
                    for (h0, h1, Xh) in ((0, lo_ch, Xlo), (lo_ch, GC, Xhi)):
                        hs = slice(h0, h1)
                        xs = slice(0, h1 - h0)
                        nh = h1 - h0
                        nc.vector.tensor_tensor(out=ew[:, hs, :], in0=Xh[:, xs, 256:260],
                                                in1=ere[:, hs, :], op=mybir.AluOpType.add)
                        nc.scalar.activation(out=w1[:, hs, :], in_=ew[:, hs, :],
                                             func=mybir.ActivationFunctionType.Exp)
                        nc.scalar.activation(out=w2[:, hs, :], in_=ew[:, hs, :],
                                             func=mybir.ActivationFunctionType.Exp, scale=NEG)
                        nc.vector.tensor_tensor(out=wb[:, hs, :], in0=w1[:, hs, :],
                                                in1=w2[:, hs, :], op=mybir.AluOpType.max)
                        # fold w into features in place ((f,h)-packed broadcast)
                        nc.vector.tensor_tensor(
                            out=Xh[:, xs, 0:256].rearrange("p j (f h) -> p j f h", f=F),
                            in0=Xh[:, xs, 0:256].rearrange("p j (f h) -> p j f h", f=F),
                            in1=wb[:, hs, None, :].broadcast_to([P, nh, F, H]),
                            op=mybir.AluOpType.mult)
                        nc.vector.tensor_copy(out=Xh[:, xs, 256:260], in_=wb[:, hs, :])
                    # chunk -> window-slot ownership per section order
                    own = []
                    for (wv, kind, ncols) in secs:
                        own += [grp.index(wv)] * ncols
                    first = {i: own.index(i) for i in range(W)}
                    last = {i: GC - 1 - own[::-1].index(i) for i in range(W)}
                    psws = []
                    for _pi in range(W):
                        psw_t = pwp.tile([P, GROW_USED], F32, tag="psw")
                        psws.append(psw_t)
                    for c in range(GC):
                        o = own[c]
                        rhs = Xlo[:, c, :] if c < lo_ch else Xhi[:, c - lo_ch, :]
                        nc.vector.tensor_scalar(out=sel[:, c, :], in0=iota_t[:],
                                                scalar1=dstf_t[:, gbase + c:gbase + c + 1],
                                                scalar2=None,
                                                op0=mybir.AluOpType.is_equal)
                        nc.tensor.matmul(out=psws[o][:], lhsT=sel[:, c, :], rhs=rhs,
                                         start=(c == first[o]), stop=(c == last[o]),
                                         skip_group_check=True)
                    # ---- phase C (per window in group): normalize, out-linear, stage x ----
                    for wi_, wv in enumerate(grp):
                        psw = psws[wi_]
                        zs = pc.tile([P, 4], F32, tag="zs")
                        nc.vector.tensor_scalar(out=zs[:], in0=psw[:, 256:260],
                                                scalar1=1e-30, scalar2=None,
                                                op0=mybir.AluOpType.max)
                        zr = pc.tile([P, 4], F32, tag="zr")
                        nc.vector.reciprocal_approx_fast(out=zr[:], in_=zs[:])
                        rstn = pc.tile([P, HF], F32, tag="rstn")
                        nc.vector.tensor_tensor(out=rstn[:].rearrange("p (f h) -> p f h", f=F),
                                                in0=psw[:, 0:256].rearrange("p (f h) -> p f h", f=F),
                                                in1=zr[:, None, :].broadcast_to([P, F, H]),
                                                op=mybir.AluOpType.mult)
                        psx = pcp.tile([P, F], F32, tag="psx")
                        for half in range(2):
                            pst = pcp.tile([P, P], F32, tag="pst")
                            nc.tensor.transpose(out=pst[:], in_=rstn[:, half * P:(half + 1) * P],
                                                identity=ident[:])
                            rT = pc.tile([P, P], F32, tag="rT")
                            nc.scalar.copy(out=rT[:], in_=pst[:])
                            nc.tensor.matmul(out=psx[:], lhsT=rT[:], rhs=outw_t[:, half, :],
                                             start=(half == 0), stop=(half == 1))
                        nc.scalar.copy(out=xall[:, wv, :], in_=psx[:])

                def do_ln(pd, w0, w1):
                    nw = w1 - w0
                    xs = xall[:, w0:w1, :]
                    nc.vector.tensor_tensor(
                        out=xs, in0=xs,
                        in1=vecs_t[:, 0:1, :].broadcast_to([P, nw, F]),
                        op=mybir.AluOpType.add)
                    s1 = pd.tile([P, nw], F32, tag="s1")
                    nc.vector.tensor_reduce(out=s1[:], in_=xs,
                                            axis=mybir.AxisListType.X,
                                            op=mybir.AluOpType.add)
                    negmu = pd.tile([P, nw], F32, tag="negmu")
                    nc.vector.tensor_scalar(out=negmu[:], in0=s1[:], scalar1=-1.0 / F,
                                            scalar2=None, op0=mybir.AluOpType.mult)
                    nc.vector.tensor_tensor(
                        out=xs, in0=xs,
                        in1=negmu[:, :, None].broadcast_to([P, nw, F]),
                        op=mybir.AluOpType.add)
                    sq = pd.tile([P, nw, F], F32, tag="sq")
                    nc.vector.tensor_tensor(out=sq[:], in0=xs, in1=xs,
                                            op=mybir.AluOpType.mult)
                    ss = pd.tile([P, nw], F32, tag="ss")
                    nc.vector.tensor_reduce(out=ss[:], in_=sq[:],
                                            axis=mybir.AxisListType.X,
                                            op=mybir.AluOpType.add)
                    v = pd.tile([P, nw], F32, tag="v")
                    nc.vector.tensor_scalar(out=v[:], in0=ss[:], scalar1=1.0 / F,
                                            scalar2=LN_EPS, op0=mybir.AluOpType.mult,
                                            op1=mybir.AluOpType.add)
                    sv = pd.tile([P, nw], F32, tag="sv")
                    nc.scalar.activation(out=sv[:], in_=v[:],
                                         func=mybir.ActivationFunctionType.Sqrt)
                    rstd = pd.tile([P, nw], F32, tag="rstd")
                    nc.vector.reciprocal_approx_fast(out=rstd[:], in_=sv[:])
                    nc.vector.tensor_tensor(
                        out=xs, in0=xs,
                        in1=rstd[:, :, None].broadcast_to([P, nw, F]),
                        op=mybir.AluOpType.mult)
                    nc.vector.tensor_tensor(
                        out=xs, in0=xs,
                        in1=vecs_t[:, 1:2, :].broadcast_to([P, nw, F]),
                        op=mybir.AluOpType.mult)
                    nc.vector.tensor_tensor(
                        out=xs, in0=xs,
                        in1=vecs_t[:, 2:3, :].broadcast_to([P, nw, F]),
                        op=mybir.AluOpType.add)
                    nc.sync.dma_start(
                        out=out_p[w0 * P:w1 * P, :].rearrange("(w p) f -> p w f", p=P),
                        in_=xs)

                ln_done = 0
                with tc.tile_pool(name="phd", bufs=2) as pd:
                    for gi in range(len(geoms) + PRE):
                        if gi < len(geoms):
                            emit_lo_er(gi)
                        if gi >= PRE:
                            j = gi - PRE
                            complete(j)
                            wdone = max(geoms[j]['grp']) + 1
                            if wdone - ln_done >= 16 or j == len(geoms) - 1:
                                do_ln(pd, ln_done, wdone)
                                ln_done = wdone
            for p in reversed(pools_b):
                p.__exit__(None, None, None)

    nc.compile()
    return nc


# ---------------- host side ----------------
def host_prep(h, src, dst, fc_w, attn_l, attn_r, gat_bias, out_w, out_b, ln_g, ln_b):
    h = np.ascontiguousarray(np.asarray(h, np.float32))
    src = np.asarray(src, np.int64)
    dst = np.asarray(dst, np.int64)
    fc_w = np.asarray(fc_w, np.float32)
    attn_l = np.asarray(attn_l, np.float32)
    attn_r = np.asarray(attn_r, np.float32)
    gat_bias = np.asarray(gat_bias, np.float32)
    out_w = np.asarray(out_w, np.float32)
    out_b = np.asarray(out_b, np.float32)
    ln_g = np.asarray(ln_g, np.float32)
    ln_b = np.asarray(ln_b, np.float32)

    A_l = np.einsum('khf,hf->kh', fc_w.reshape(F, H, F), attn_l).astype(np.float32)
    A_r = np.einsum('khf,hf->kh', fc_w.reshape(F, H, F), attn_r).astype(np.float32)
    # fc cols permuted to (f,h)-major so phase A writes G rows without a permute
    perm = (np.arange(F)[:, None] + np.arange(H)[None, :] * F).reshape(-1)
    Waug = np.ascontiguousarray(np.concatenate([fc_w[:, perm], A_l, A_r], axis=1))  # [64, 264]
    bias2 = (gat_bias @ out_w + out_b).astype(np.float32)                  # [64]
    # out_w permuted to (f,h)-major rows to match the G feat layout
    outw_perm = np.ascontiguousarray(
        out_w.reshape(H, F, F).transpose(1, 0, 2).reshape(HF, F))

    import ml_dtypes
    hT = np.zeros((F, GROWS), ml_dtypes.bfloat16)
    hT[:, :N] = h.T.astype(ml_dtypes.bfloat16)
    hTo = np.zeros((NCORES, F, ERROWS), ml_dtypes.bfloat16)
    for c in range(NCORES):
        hTo[c, :, :NPC] = h[c * NPC:(c + 1) * NPC].T.astype(ml_dtypes.bfloat16)

    vecs = np.zeros((P, 3, F), np.float32)
    vecs[:, 0, :] = bias2
    vecs[:, 1, :] = ln_g
    vecs[:, 2, :] = ln_b

    # sort edges by dst
    order = np.argsort(dst, kind='stable')
    ssrc = src[order]
    sdst = dst[order]
    core_of = sdst // NPC
    loc = sdst - core_of * NPC
    win = loc // P
    gw = core_of * NWIN + win
    counts = np.bincount(gw, minlength=NCORES * NWIN)
    starts = np.zeros(NCORES * NWIN + 1, np.int64)
    np.cumsum(counts, out=starts[1:])
    lomask = ssrc < LO

    def pack_section(e_src, e_loc, wv):
        """Pack a section's edges into pair slots. Pair t = (p=t%P, blk=t//P)
        covers cols (2blk, 2blk+1); consecutive dst-sorted edges pair up when
        the 2nd edge's local row is r0 or r0+1 (one 16B table row covers both).
        Returns (npairs, lists-of (pair_t, b, src, dloc, mk, eridx))."""
        ne = len(e_src)
        slots = []
        i, t = 0, 0
        while i < ne:
            r0 = int(e_loc[i])
            slots.append((t, 0, int(e_src[i]), int(e_loc[i]) - wv * P, 0.0, r0 + 1))
            i += 1
            if i < ne:
                r1 = int(e_loc[i])
                if r1 == r0 or r1 == r0 + 1:
                    slots.append((t, 1, int(e_src[i]), r1 - wv * P,
                                  float(r1 - r0), r0 + 1))
                    i += 1
            t += 1
        return t, slots

    # first pass: pack and derive static per-window chunk counts (max / cores)
    packed = {}
    CLw = np.ones(NWIN, np.int64)
    CHw = np.ones(NWIN, np.int64)
    for c in range(NCORES):
        for w in range(NWIN):
            g = c * NWIN + w
            sl = slice(starts[g], starts[g + 1])
            m = lomask[sl]
            s_src = ssrc[sl]
            s_loc = loc[sl]
            for kind, es, el_ in (('lo', s_src[m], s_loc[m]),
                                  ('hi', s_src[~m] - LO, s_loc[~m])):
                npairs, slots = pack_section(es, el_, w)
                packed[(c, w, kind)] = (npairs, slots)
                need = 1
                for (t, b, _s, _d, _m, _e) in slots:
                    need = max(need, 2 * (t // P) + b + 1)
                if kind == 'lo':
                    CLw[w] = max(CLw[w], need)
                else:
                    CHw[w] = max(CHw[w], need)

    geoms, TOTC, TOTJ = _geom(CLw, CHw)

    srcq = np.zeros((NCORES, P, TOTC * 8), np.int16)
    erq = np.ones((NCORES, P, TOTJ * 8), np.int16)
    maskq = np.zeros((NCORES, P, TOTC, 4), np.int8)
    dstf = np.full((NCORES, P, TOTC), 200.0, np.float32)

    def tile16(a):
        # idx layout for the gather: [16, n*8] wrapped, replicated to 128 rows
        ncols = a.shape[0]
        return np.tile(a.reshape(ncols * 8, 16).T, (8, 1))

    for c in range(NCORES):
        for gm in geoms:
            colbase, jcolbase = gm['colbase'], gm['jcolbase']
            c0, j0 = 0, 0
            for (wv, kind, ncols) in gm['secs']:
                nb = ncols // 2
                tail = ncols % 2
                npairs, slots = packed[(c, wv, kind)]
                sq = np.zeros((ncols, P), np.int16)
                df = np.full((ncols, P), 200.0, np.float32)
                mk = np.zeros((ncols, P), np.float32)
                ej = np.ones((nb + tail, P), np.int16)
                for (t, b, s_, dl, m_, er_) in slots:
                    p_ = t % P
                    blk = t // P
                    col = 2 * blk + b
                    assert col < ncols, (c, wv, kind, col, ncols)
                    sq[col, p_] = s_
                    df[col, p_] = dl
                    mk[col, p_] = m_
                    jc = blk if (2 * blk + 1 < ncols) else nb  # tail jcol
                    if 2 * blk + 1 >= ncols:
                        assert b == 0
                    ej[jc, p_] = er_
                cb = colbase + c0
                jb = jcolbase + j0
                srcq[c][:, cb * 8:(cb + ncols) * 8] = tile16(sq)
                erq[c][:, jb * 8:(jb + nb + tail) * 8] = tile16(ej)
                dstf[c][:, cb:cb + ncols] = df.T
                maskq[c][:, cb:cb + ncols, :] = mk.T[:, :, None]
                c0 += ncols
                j0 += nb + tail

    small = dict(Waug=Waug.astype(ml_dtypes.bfloat16), outw=outw_perm, vecs=vecs)
    return hT, hTo, srcq, erq, maskq, dstf, small, tuple(CLw), tuple(CHw)


_prog_cache = {}

def kernel(**inputs):
    hT, hTo, srcq, erq, maskq, dstf, small, CLw, CHw = host_prep(**inputs)
    key = (CLw, CHw)
    if key not in _prog_cache:
        _prog_cache[key] = build_program(CLw, CHw)
    nc = _prog_cache[key]
    in_maps = []
    for c in range(NCORES):
        in_maps.append({
            "hT": hT, "hTo": hTo[c], "Waug": small["Waug"], "outw": small["outw"],
            "vecs": small["vecs"], "srcq": srcq[c], "erq": erq[c],
            "mask": maskq[c], "dstf": dstf[c],
        })
    res = run_bass_kernel_spmd(nc, in_maps, list(range(NCORES)))
    out = np.concatenate([np.asarray(res.results[c]["out"])[:NPC] for c in range(NCORES)], axis=0)
    return out
